# revision 33
# baseline (speedup 1.0000x reference)
"""ArcFace loss kernel for 8 TRN2 NeuronCores (Bass, SPMD class-parallel).

Design (class/tensor parallel per the sharding hint):
  - weight [100000, 512] f32 sharded over classes: 12500/core, zero-padded to
    12544 = 98 tiles of 128; pad rows contribute exactly exp(0)=1 to the
    softmax sum and are subtracted out as a constant after the all-reduce.
  - Per 2-tile group (49 groups/core), a 7-stage software pipeline
    (one stage per "wave"; all buffers are rings sized to the span):
      wave g+0  DVE : fused square+row-sum (scalar_tensor_tensor accum) ->
                      ||w_c||^2 from the bf16 weights
      wave g+1  ACT : scale_c = 64/||w_c|| via Ln+Exp (stays inside the
                      natural_log_exp table set; Sqrt/Rsqrt never loaded)
      wave g+2  PE  : transpose via 8 regular bf16 matmuls against a
                      host-passed identity (w tile as stationary operand)
      wave g+3  DVE+ACT : PSUM -> SBUF bf16 copies of the transposed tile
      wave g+4  PE  : cosine matmul [class-part, batch-free], 8 matmuls
                      accumulating d-chunks into PSUM
      wave g+5  ACT : exp(scale_c * psum) with PER-PARTITION scale AP --
                      normalization is folded into the activation, and the
                      exp uses no max-shift (logits bounded by +-64 fit f32;
                      keeps ln(S) in the ACT table's accurate range)
      wave g+6  DVE : bf16 accumulation of exp tiles
  - Weight stream: SWDGE DMA with inline f32->bf16 cast, 0.5MB per group,
    6-deep prefetch ring (~25.6MB/core total HBM traffic = the roofline).
  - S[512] = ones-matmul partition-reduction of the accumulator; ONE tiny
    AllReduce(add) over [512] f32 combines the 8 cores (the fixed-shift exp
    eliminated the usual max all-reduce entirely).
  - ArcFace margin only affects each row's target logit: every core
    redundantly computes cos_theta for the 512 host-gathered target rows
    (weight[y] - pure indexing), applies the margin formula with an
    arithmetic select, and corrects S by exp(64*m(c)) - exp(64*c).
  - loss = mean_b(ln(S_true_b) - 64*m(cos_t_b)), finished on device.

Scheduling: static per-engine instruction streams with precomputed
semaphore targets (event table), verified race-free in MultiCoreSim.
"""

import sys

if "/opt/trn_rl_repo" not in sys.path:
    sys.path.insert(0, "/opt/trn_rl_repo")

import math
from contextlib import ExitStack

import numpy as np
import ml_dtypes

import concourse.bass as bass
import concourse.mybir as mybir
from concourse.bass_utils import run_bass_kernel_spmd

# ---- problem constants (hardcoded; kernel.py must be self-contained) ----
B = 512          # batch
D = 512          # feature dim
C = 100000       # num classes
N_CORES = 8
C_SHARD = C // N_CORES          # 12500
TILE = 128                      # classes per tile
GT = 2                          # tiles per group
N_TILES_FULL = 98               # ceil(12500/128) = 98 -> 12544 padded
C_PAD = N_TILES_FULL * TILE     # 12544

M_MARGIN = 0.5
SCALE = 64.0
COS_M = float(np.cos(M_MARGIN))
SIN_M = float(np.sin(M_MARGIN))
TH = -COS_M
MM = SIN_M * M_MARGIN
LN64 = float(np.log(64.0))
MAXV = 64.0   # fixed logsumexp shift

F32 = mybir.dt.float32
BF16 = mybir.dt.bfloat16
ADD = mybir.AluOpType.add
SUB = mybir.AluOpType.subtract
MUL = mybir.AluOpType.mult
AF = mybir.ActivationFunctionType


class Sched:
    """Static schedule: per-engine op lists with symbolic event waits.

    Two passes: first assign each event its cumulative semaphore value in
    engine program order, then emit per-engine instruction streams with
    resolved wait_ge targets.
    """

    def __init__(self):
        self.ops = {k: [] for k in ("gpsimd", "vector", "scalar", "tensor", "sync")}
        self.ev = {}  # event name -> (sem_key, value)

    def add(self, engine, waits, emit, inc_event=None, inc_amount=1):
        self.ops[engine].append((list(waits), emit, inc_event, inc_amount))

    def assign(self, sem_key_of_engine):
        counts = {}
        for eng, ops in self.ops.items():
            sem_key = sem_key_of_engine[eng]
            for (_w, _e, ev, amt) in ops:
                if ev is not None:
                    counts[sem_key] = counts.get(sem_key, 0) + amt
                    assert ev not in self.ev, ev
                    self.ev[ev] = (sem_key, counts[sem_key])


def build_nc(n_tiles=N_TILES_FULL, core_ids=None, debug=False, repeat=1,
             no_collective=False, ablate=(), timing_mode=False):
    """Build the SPMD Bass graph. Returns nc."""
    assert n_tiles % GT == 0
    NDATA = n_tiles // GT          # distinct data groups
    NG = NDATA * repeat            # loop iterations (re-reads data when >1)
    c_pad = n_tiles * TILE
    if core_ids is None:
        core_ids = list(range(N_CORES))

    nc = bass.Bass()

    # register the float constants used as activation biases
    for cv in (-MAXV, LN64, 1e-24):
        t = nc.alloc_sbuf_tensor(f"const-f32-{cv}", [128, 1], F32)
        nc.gpsimd.memset(t.ap(), cv)
        nc.const_aps.aps[(F32, cv)] = t.ap()
    nc.all_engine_barrier()

    x_ext = nc.declare_dram_parameter("x", [B, D], F32, isOutput=False)
    if timing_mode:
        # timing-only: weight lives in internal DRAM (uninitialized garbage)
        # so per-call host->device traffic is tiny and device time is
        # measurable through the dispatch pipeline.
        w_ext = nc.dram_tensor("w_internal", [c_pad, D], F32)
    else:
        w_ext = nc.declare_dram_parameter("w", [c_pad, D], F32, isOutput=False)
    wg_ext = nc.declare_dram_parameter("wg", [B, D], F32, isOutput=False)
    id_ext = nc.declare_dram_parameter("ident", [128, 128], BF16, isOutput=False)
    out_ext = nc.declare_dram_parameter("out", [1, 1], F32, isOutput=True)
    if debug:
        dbg_ext = nc.declare_dram_parameter("dbg", [6, 512], F32, isOutput=True)

    in_cc = nc.dram_tensor("in_cc", [1, B], F32)
    out_cc = nc.dram_tensor("out_cc", [1, B], F32, addr_space="Shared")

    ctx = ExitStack()
    sb = lambda name, shape, dt: ctx.enter_context(nc.sbuf_tensor(name, shape, dt))
    ps = lambda name, shape, dt: ctx.enter_context(nc.psum_tensor(name, shape, dt))

    # ---- SBUF ----
    ident = sb("ident_sb", [128, 128], BF16)
    xf = sb("xf", [128, 4 * D], F32)          # x, b-tile-major [p, (t,d)]
    wg_sb = sb("wg_sb", [128, 4 * D], F32)    # gathered target rows, same layout
    xn_bf = sb("xn_bf", [128, 4 * D], BF16)   # normalized x (bf16)
    xnT = sb("xnT", [128, 4 * B], BF16)       # x^T chunks [p=d, (j,b)]
    WR = 6                                      # wbuf ring depth (waves)
    wbuf = sb("wbuf", [128, WR * GT * D], BF16)  # w natives ring
    wTn = sb("wTn", [128, 2 * GT * 4 * 128], BF16)  # normalized transposed w
    wsq = sb("wsq", [128, 4 * GT * D], BF16)  # ttr dump ring (4 waves)
    xsq = sb("xsq", [128, 8 * D], F32)        # prep ttr dump, 8 slots
    exp_sb = sb("exp_sb", [128, 2 * GT * B], BF16)  # exp output, 2 slots
    acc = sb("acc", [128, GT * B], BF16)      # exp accumulator
    NR, SR = 4, 6                             # norm2 / scale64 ring depths
    norm2 = sb("norm2", [128, NR * GT], F32)
    lnq = sb("lnq", [128, SR * GT], F32)
    scale64 = sb("scale64", [128, SR * GT], F32)
    ones_bf = sb("ones_bf", [128, 1], BF16)
    ones_f32 = sb("ones_f32", [128, 1], F32)
    S_sb = sb("S_sb", [1, B], F32)
    S_ar = sb("S_ar", [128, 4], F32)
    loss_sb = sb("loss_sb", [1, 1], F32)
    # small target-term tensors [128, 4] (b = 128*t + p)
    names = ["xq", "xlnq", "xscale", "dots", "qwg", "wglnq", "rnw", "ct0",
             "cos_t", "c2", "om", "omc", "lnom", "sin_t", "sinSM", "cosm",
             "cmm", "maskt", "mt", "diffm", "mdm", "e_plain", "e_marg", "delta", "tm64",
             "S_true", "lnS", "lv0", "lossvec", "junk4"]
    small = {n: sb(n, [128, 4], F32) for n in names}
    (xq, xlnq, xscale, dots, qwg, wglnq, rnw, ct0, cos_t, c2, om, omc, lnom,
     sin_t, sinSM, cosm, cmm, maskt, mt, diffm, mdm, e_plain, e_marg, delta,
     tm64, S_true, lnS, lv0, lossvec, junk4) = (small[n] for n in names)
    loss_col = sb("loss_col", [128, 1], F32)

    # ---- PSUM ----  (psumT: 2 slots x 2 banks; pmA/pmB: 2 banks each)
    psumT = ps("psumT", [128, 2048], F32)
    pmA = ps("pmA", [128, GT * B], F32)
    pmB = ps("pmB", [128, GT * B], F32)
    pm = [pmA, pmB]

    sched = Sched()

    # chain sems give the race detector an explicit edge between dependent
    # instructions on the SAME engine (HW executes them in order anyway).
    chain_state = {"vector": 0, "scalar": 0}
    chain_sems = {}

    def chain(eng_obj, eng_name, instr):
        chain_state[eng_name] += 1
        instr.then_inc(chain_sems[eng_name], 1)
        eng_obj.wait_ge(chain_sems[eng_name], chain_state[eng_name])

    # ---------------- gpsimd: DMAs + collective ----------------
    def dma_ident(g):
        return g.dma_start(out=ident[:], in_=id_ext[:])

    def dma_x(g):
        return g.dma_start(
            out=xf[:].rearrange("p (t d) -> p t d", t=4),
            in_=x_ext[:].rearrange("(t p) d -> p t d", p=128))

    def dma_wg(g):
        return g.dma_start(
            out=wg_sb[:].rearrange("p (t d) -> p t d", t=4),
            in_=wg_ext[:].rearrange("(t p) d -> p t d", p=128))

    sched.add("gpsimd", [], dma_ident, ("dm0", "ident"), 16)
    sched.add("gpsimd", [], dma_x, ("dm1", "x"), 16)
    sched.add("gpsimd", [], dma_wg, ("dm2", "wg"), 16)

    def dma_w(gi):
        def f(g):
            di = gi % NDATA
            if "dma" in ablate:
                return g.dma_start(out=wbuf[:, 0:4],
                                   in_=w_ext[di * GT * TILE:di * GT * TILE + 128, 0:4])
            rows = w_ext[di * GT * TILE:(di + 1) * GT * TILE, :]
            return g.dma_start(
                out=wbuf[:, (gi % WR) * GT * D:(gi % WR + 1) * GT * D].rearrange(
                    "p (T d) -> p T d", T=GT),
                in_=rows.rearrange("(T p) d -> p T d", p=128))
        return f

    for gi in range(min(WR, NG)):
        sched.add("gpsimd", [], dma_w(gi), ("dw%d" % (gi % WR), ("w", gi)), 16)

    # ---------------- DVE stream ----------------
    def dve_init(v):
        v.memset(ones_bf[:], 1.0)
        v.memset(ones_f32[:], 1.0)
        return v.memset(acc[:], 0.0)

    sched.add("vector", [], dve_init, ("dve", "init"), 1)

    def xttr(v):
        last = None
        for t in range(4):
            last = v.scalar_tensor_tensor(
                out=xsq[:, t * D:(t + 1) * D], in0=xf[:, t * D:(t + 1) * D],
                scalar=1.0, in1=xf[:, t * D:(t + 1) * D],
                op0=MUL, op1=MUL, accum_out=xq[:, t:t + 1])
        return last

    sched.add("vector", [("dm1", "x")], xttr, ("dve", "xq"), 1)

    def xnmul(v):
        last = None
        for t in range(4):
            last = v.tensor_scalar(
                out=xn_bf[:, t * D:(t + 1) * D], in0=xf[:, t * D:(t + 1) * D],
                scalar1=xscale[:, t:t + 1], scalar2=None, op0=MUL)
        return last

    sched.add("vector", [("act", "xscale")], xnmul, ("dve", "xn"), 1)

    def wgttr(v):
        last = None
        for t in range(4):
            v.scalar_tensor_tensor(
                out=xsq[:, t * D:(t + 1) * D], in0=xf[:, t * D:(t + 1) * D],
                scalar=1.0, in1=wg_sb[:, t * D:(t + 1) * D],
                op0=MUL, op1=MUL, accum_out=dots[:, t:t + 1])
        for t in range(4):
            last = v.scalar_tensor_tensor(
                out=xsq[:, (4 + t) * D:(5 + t) * D],
                in0=wg_sb[:, t * D:(t + 1) * D], scalar=1.0,
                in1=wg_sb[:, t * D:(t + 1) * D],
                op0=MUL, op1=MUL, accum_out=qwg[:, t:t + 1])
        return last

    sched.add("vector", [("dm2", "wg"), ("dve", "xq")], wgttr, ("dve", "wgq"), 1)

    def coschain(v):
        chain(v, "vector", v.tensor_tensor(ct0[:], dots[:], xscale[:], MUL))
        chain(v, "vector", v.tensor_tensor(cos_t[:], ct0[:], rnw[:], MUL))
        chain(v, "vector", v.tensor_tensor(c2[:], cos_t[:], cos_t[:], MUL))
        chain(v, "vector", v.tensor_scalar(out=om[:], in0=c2[:], scalar1=-1.0,
                                           scalar2=1.0, op0=MUL, op1=ADD))
        return v.tensor_scalar_max(omc[:], om[:], 1e-30)

    sched.add("vector", [("act", "rnw")], coschain, ("dve", "om"), 1)

    def mtchain(v):
        chain(v, "vector", v.tensor_scalar_mul(sinSM[:], sin_t[:], SIN_M))
        chain(v, "vector", v.scalar_tensor_tensor(
            out=cosm[:], in0=cos_t[:], scalar=COS_M, in1=sinSM[:],
            op0=MUL, op1=SUB))
        chain(v, "vector", v.tensor_scalar_add(cmm[:], cos_t[:], -MM))
        chain(v, "vector", v.tensor_scalar(
            out=maskt[:], in0=cos_t[:], scalar1=TH, scalar2=None,
            op0=mybir.AluOpType.is_gt))
        chain(v, "vector", v.tensor_tensor(diffm[:], cosm[:], cmm[:], SUB))
        chain(v, "vector", v.tensor_tensor(mdm[:], maskt[:], diffm[:], MUL))
        return v.tensor_tensor(mt[:], cmm[:], mdm[:], ADD)

    sched.add("vector", [("act", "sin")], mtchain, ("dve", "mt"), 1)

    pad_total = float(N_CORES * (c_pad - min(C_SHARD, c_pad)))

    def tgtfin(v):
        chain(v, "vector", v.tensor_tensor(delta[:], e_marg[:], e_plain[:], SUB))
        chain(v, "vector", v.tensor_scalar_add(lv0[:], delta[:], -pad_total))
        return v.tensor_scalar_mul(tm64[:], mt[:], 64.0)

    sched.add("vector", [("act", "etp")], tgtfin, ("dve", "tgt"), 1)

    # group-loop DVE
    def d_norm(gi):
        def f(v):
            if "norm" in ablate:
                return v.memset(norm2[:, (gi % NR) * GT:(gi % NR) * GT + GT], 1.0)
            last = None
            for t in range(GT):
                wl = slice(((gi % WR) * GT + t) * D, ((gi % WR) * GT + t + 1) * D)
                dl = slice(((gi % 4) * GT + t) * D, ((gi % 4) * GT + t + 1) * D)
                last = v.scalar_tensor_tensor(
                    out=wsq[:, dl], in0=wbuf[:, wl], scalar=1.0,
                    in1=wbuf[:, wl], op0=MUL, op1=MUL,
                    accum_out=norm2[:, (gi % NR) * GT + t:(gi % NR) * GT + t + 1])
            return last
        return f

    def d_copy(gi):
        def f(v):
            pbase = (gi % 2) * 1024
            if "copy" in ablate:
                return v.tensor_copy(
                    wTn[:, (gi % 2) * 1024:(gi % 2) * 1024 + 4],
                    psumT[:, pbase:pbase + 4])
            return v.tensor_copy(
                wTn[:, (gi % 2) * 1024:(gi % 2) * 1024 + 512],
                psumT[:, pbase:pbase + 512])
        return f

    def d_acc(gi):
        def f(v):
            if "acc" in ablate:
                return v.tensor_tensor(
                    acc[:, 0:4], acc[:, 0:4],
                    exp_sb[:, (gi % 2) * GT * B:(gi % 2) * GT * B + 4], ADD)
            return v.tensor_tensor(
                acc[:], acc[:],
                exp_sb[:, (gi % 2) * GT * B:(gi % 2 + 1) * GT * B], ADD)
        return f



    # ---------------- ACT stream ----------------
    def a_xscale(a):
        chain(a, "scalar", a.activation(xlnq[:], xq[:], AF.Ln, bias=1e-24))
        return a.activation(xscale[:], xlnq[:], AF.Exp, scale=-0.5)

    sched.add("scalar", [("dve", "xq")], a_xscale, ("act", "xscale"), 1)

    # xnT transpose copies (interleaved with TE xT matmuls)
    def a_xTc(j):
        def f(a):
            return a.activation(
                xnT[:, j * B:(j + 1) * B], psumT[:, (j % 2) * 512:(j % 2) * 512 + 512],
                AF.Copy)
        return f

    for j in range(4):
        sched.add("scalar", [("te", ("xT", j))], a_xTc(j), ("act", ("xTc", j)), 1)

    def a_rnw(a):
        chain(a, "scalar", a.activation(wglnq[:], qwg[:], AF.Ln, bias=1e-24))
        return a.activation(rnw[:], wglnq[:], AF.Exp, scale=-0.5)

    sched.add("scalar", [("dve", "wgq")], a_rnw, ("act", "rnw"), 1)

    def a_sin(a):
        chain(a, "scalar", a.activation(lnom[:], omc[:], AF.Ln))
        return a.activation(sin_t[:], lnom[:], AF.Exp, scale=0.5)

    sched.add("scalar", [("dve", "om")], a_sin, ("act", "sin"), 1)

    def a_etp(a):
        a.activation(e_plain[:], cos_t[:], AF.Exp, scale=64.0)
        return a.activation(e_marg[:], mt[:], AF.Exp, scale=64.0)

    sched.add("scalar", [("dve", "mt")], a_etp, ("act", "etp"), 1)

    def a_scale(gi):
        def f(a):
            nl = slice((gi % NR) * GT, (gi % NR) * GT + GT)
            sl = slice((gi % SR) * GT, (gi % SR) * GT + GT)
            if "scale" in ablate:
                return a.activation(scale64[:, sl], norm2[:, nl], AF.Copy)
            chain(a, "scalar", a.activation(lnq[:, sl], norm2[:, nl], AF.Ln, bias=1e-24))
            return a.activation(scale64[:, sl], lnq[:, sl], AF.Exp,
                                scale=-0.5, bias=LN64)
        return f

    def a_copy(gi):
        def f(a):
            pbase = (gi % 2) * 1024
            if "copy" in ablate:
                return a.activation(
                    wTn[:, (gi % 2) * 1024 + 512:(gi % 2) * 1024 + 516],
                    psumT[:, pbase + 512:pbase + 516], AF.Copy)
            return a.activation(
                wTn[:, (gi % 2) * 1024 + 512:(gi % 2) * 1024 + 1024],
                psumT[:, pbase + 512:pbase + 1024], AF.Copy)
        return f

    def a_exp(gi):
        def f(a):
            if "exp" in ablate:
                return a.activation(
                    exp_sb[:, (gi % 2) * GT * B:(gi % 2) * GT * B + 4],
                    pm[gi % 2][:, 0:4], AF.Exp, scale=1.0)
            last = None
            for t in range(GT):
                last = a.activation(
                    exp_sb[:, ((gi % 2) * GT + t) * B:((gi % 2) * GT + t + 1) * B],
                    pm[gi % 2][:, t * B:(t + 1) * B], AF.Exp,
                    scale=scale64[:, (gi % SR) * GT + t:(gi % SR) * GT + t + 1])
            return last
        return f



    # ---------------- TensorE stream ----------------
    def t_xT(j):
        def f(te):
            last = None
            for t in range(4):
                last = te.matmul(
                    psumT[:, (j % 2) * 512 + t * 128:(j % 2) * 512 + (t + 1) * 128],
                    lhsT=xn_bf[:, t * D + j * 128:t * D + (j + 1) * 128],
                    rhs=ident[:], start=True, stop=True)
            return last
        return f

    for j in range(4):
        w = [("dve", "xn"), ("dm0", "ident")]
        if j >= 2:
            w.append(("act", ("xTc", j - 2)))
        sched.add("tensor", w, t_xT(j), ("te", ("xT", j)), 1)

    def t_T(gi):
        def f(te):
            if "tmm" in ablate:
                return te.matmul(psumT[:, 0:4],
                                 lhsT=wbuf[:, 0:128], rhs=ident[:, 0:4],
                                 start=True, stop=True)
            last = None
            pbase = (gi % 2) * 1024
            for t in range(GT):
                for j in range(4):
                    last = te.matmul(
                        psumT[:, pbase + (t * 4 + j) * 128:
                              pbase + (t * 4 + j + 1) * 128],
                        lhsT=wbuf[:, ((gi % WR) * GT + t) * D + j * 128:
                                  ((gi % WR) * GT + t) * D + (j + 1) * 128],
                        rhs=ident[:], start=True, stop=True)
            return last
        return f

    def t_M(gi):
        def f(te):
            if "mmm" in ablate:
                return te.matmul(pm[gi % 2][:, 0:4],
                                 lhsT=wTn[:, 0:128], rhs=xnT[:, 0:4],
                                 start=True, stop=True)
            last = None
            for t in range(GT):
                for j in range(4):
                    last = te.matmul(
                        pm[gi % 2][:, t * B:(t + 1) * B],
                        lhsT=wTn[:, (gi % 2) * 1024 + (t * 4 + j) * 128:
                                 (gi % 2) * 1024 + (t * 4 + j + 1) * 128],
                        rhs=xnT[:, j * B:(j + 1) * B],
                        start=(j == 0), stop=(j == 3))
            return last
        return f

    # Software-pipelined schedule with per-stage events:
    #   norm@g  scale@g+1  T@g+2  copy@g+3  M@g+4  exp@g+5  acc@g+6
    for w in range(NG + 6):
        g = w
        if g < NG:   # norm(g)
            w1 = [("dw%d" % (g % WR), ("w", g))]
            if g >= 4:
                w1.append(("dve", ("norm", g - 4)))   # wsq ring reuse (self)
            if g >= NR:
                w1.append(("act", ("scale", g - NR)))  # norm2 ring reuse
            sched.add("vector", w1, d_norm(g), ("dve", ("norm", g)), 1)
        g = w - 1
        if 0 <= g < NG:  # scale(g)
            ws = [("dve", ("norm", g))]
            if g >= SR:
                ws.append(("act", ("exp", g - SR)))   # scale64 ring reuse (self)
            sched.add("scalar", ws, a_scale(g), ("act", ("scale", g)), 1)
        g = w - 2
        if 0 <= g < NG:  # T(g)
            if g >= 2:
                wl = [("dve", ("copyD", g - 2)), ("act", ("copyA", g - 2))]
            else:
                wl = [("dve", ("norm", g)), ("act", ("xTc", 3)),
                      ("dm0", "ident")]
            sched.add("tensor", wl, t_T(g), ("te", ("T", g)), 1)
        if WR <= w < NG:  # gated weight prefetch
            sched.add("gpsimd", [("te", ("T", w - WR))], dma_w(w),
                      ("dw%d" % (w % WR), ("w", w)), 16)
        g = w - 3
        if 0 <= g < NG:  # copyD(g) / copyA(g)
            wl = [("te", ("M", g - 2)) if g >= 2 else ("te", ("T", g))]
            sched.add("vector", wl, d_copy(g), ("dve", ("copyD", g)), 1)
            sched.add("scalar", list(wl), a_copy(g), ("act", ("copyA", g)), 1)
        g = w - 4
        if 0 <= g < NG:  # M(g)
            wl = [("dve", ("copyD", g))]
            if g >= 2:
                wl.append(("act", ("exp", g - 2)))   # covers copyA(g) + pm reuse
            else:
                wl.append(("act", ("copyA", g)))
            sched.add("tensor", wl, t_M(g), ("te", ("M", g)), 1)
        g = w - 5
        if 0 <= g < NG:  # exp(g)
            w3 = [("te", ("M", g))]
            if g >= 2:
                w3.append(("dve", ("acc", g - 2)))
            else:
                w3.append(("act", ("scale", g)))      # self, scale64 operand
            sched.add("scalar", w3, a_exp(g), ("act", ("exp", g)), 1)
        g = w - 6
        if 0 <= g < NG:  # acc(g)
            wacc = [("act", ("exp", g))]
            wacc.append(("dve", ("acc", g - 1)) if g >= 1 else ("dve", "init"))
            sched.add("vector", wacc, d_acc(g), ("dve", ("acc", g)), 1)

    # final: S allreduce + output
    sched.add("gpsimd", [("dve", "Scopy")],
              lambda g: g.dma_start(out=in_cc[:], in_=S_sb[:]),
              ("dm3", "incc"), 16)

    if no_collective:
        def coll(g):
            return g.dma_start(out=out_cc[:], in_=in_cc[:])
        sched.add("gpsimd", [("dm3", "incc")], coll, ("cc", "ar"), 16)
    else:
        def coll(g):
            return g.collective_compute(
                "AllReduce", ADD, replica_groups=[core_ids],
                ins=[in_cc[:]], outs=[out_cc[:]])
        sched.add("gpsimd", [("dm3", "incc")], coll, ("cc", "ar"), 1)
    def dma_sar(g):
        with nc.allow_non_contiguous_dma(reason="tiny [512] -> [128,4] reshape"):
            return g.dma_start(
                out=S_ar[:].rearrange("p (o t) -> p o t", o=1),
                in_=out_cc[:].rearrange("o (t p) -> p o t", p=128))

    sched.add("gpsimd", [("cc", "ar")], dma_sar, ("dm4", "sar"), 16)
    sched.add("gpsimd", [("act", "loss_sb")],
              lambda g: g.dma_start(out=out_ext[:], in_=loss_sb[:]),
              ("dm5", "out"), 16)
    if debug:
        def mk_dbg0(g):
            return g.dma_start(out=dbg_ext[0:1, :], in_=S_sb[:])
        sched.add("gpsimd", [("dm5", "out")], mk_dbg0, ("dm5", "dbg0"), 16)

        def mk_dbg(i, t):
            def f(g):
                with nc.allow_non_contiguous_dma(reason="debug dump"):
                    return g.dma_start(
                        out=dbg_ext[1 + i:2 + i, :].rearrange(
                            "o (t p) -> p o t", p=128),
                        in_=t[:].rearrange("p t -> p () t"))
            return f
        for i, t in enumerate((S_ar, cos_t, mt, S_true, delta)):
            sched.add("gpsimd", [("dm5", "dbg%d" % i)], mk_dbg(i, t),
                      ("dm5", "dbg%d" % (i + 1)), 16)


    # final DVE
    sched.add("vector", [("te", "S")],
              lambda v: v.tensor_copy(S_sb[:], psumT[0:1, 0:512]),
              ("dve", "Scopy"), 1)
    sched.add("vector", [("dm4", "sar"), ("dve", "tgt")],
              lambda v: v.tensor_tensor(S_true[:], S_ar[:], lv0[:], ADD),
              ("dve", "Strue"), 1)

    def losschain(v):
        chain(v, "vector", v.tensor_tensor(lossvec[:], lnS[:], tm64[:], SUB))
        return v.tensor_scalar(out=junk4[:], in0=lossvec[:], scalar1=1.0,
                               scalar2=None, op0=MUL, op1=ADD,
                               accum_out=loss_col[:])

    sched.add("vector", [("act", "lnS")], losschain, ("dve", "losscol"), 1)

    sched.add("scalar", [("dve", "Strue")],
              lambda a: a.activation(lnS[:], S_true[:], AF.Ln),
              ("act", "lnS"), 1)
    sched.add("scalar", [("te", "L")],
              lambda a: a.activation(loss_sb[:], psumT[0:1, 512:513],
                                     AF.Copy, scale=1.0 / B),
              ("act", "loss_sb"), 1)

    def t_S(te):
        last = None
        for i in range(GT):
            last = te.matmul(psumT[0:1, 0:512], lhsT=ones_bf[:],
                             rhs=acc[:, i * B:(i + 1) * B],
                             start=(i == 0), stop=(i == GT - 1))
        return last

    sched.add("tensor", [("dve", ("acc", NG - 1)),
                         ("act", ("exp", NG - 1))], t_S, ("te", "S"), 1)
    sched.add("tensor", [("dve", "losscol")],
              lambda te: te.matmul(psumT[0:1, 512:513], lhsT=ones_f32[:],
                                   rhs=loss_col[:], start=True, stop=True),
              ("te", "L"), 1)

    # ---------------- assign + emit ----------------
    sem_of = {"gpsimd": "dma", "vector": "dve", "scalar": "act",
              "tensor": "te", "sync": "sync"}
    sched.assign(sem_of)
    # collective increments a different sem than gpsimd's dma sem: fix up.
    # (Sched.assign used engine->sem mapping; the collective op was given
    # event key ("cc","ar") — recompute its value on the cc sem.)
    # Simpler: values were accumulated on "dma" for gpsimd ops including the
    # collective; patch: recompute manually below.
    sched.ev.clear()
    counts = {}
    for eng, ops in sched.ops.items():
        for (_w, _e, ev, amt) in ops:
            if ev is None:
                continue
            sem_key = ev[0]
            counts[sem_key] = counts.get(sem_key, 0) + amt
            sched.ev[ev] = (sem_key, counts[sem_key])

    sem_keys = ["dm0", "dm1", "dm2", "dm3", "dm4", "dm5",
                "dw0", "dw1", "dw2", "dw3", "dw4", "dw5",
                "dve", "act", "te", "cc", "chv", "cha", "pl"]
    semctx = ExitStack()
    sems = {k: semctx.enter_context(nc.semaphore(f"s_{k}")) for k in sem_keys}
    with nc.Block() as block:
        chain_sems["vector"] = sems["chv"]
        chain_sems["scalar"] = sems["cha"]

        def emit(engine_obj, eng_name):
            for (waits, emit_fn, ev, amt) in sched.ops[eng_name]:
                for wkey in waits:
                    sem_key, val = sched.ev[wkey]
                    engine_obj.wait_ge(sems[sem_key], val)
                instr = emit_fn(engine_obj)
                if ev is not None:
                    instr.then_inc(sems[ev[0]], amt)

        @block.gpsimd
        def _(g):
            emit(g, "gpsimd")

        @block.vector
        def _(v):
            emit(v, "vector")

        @block.scalar
        def _(a):
            emit(a, "scalar")

        @block.tensor
        def _(te):
            emit(te, "tensor")

    ctx.close()
    return nc


# ---------------- host wrapper ----------------
_cache = {}


def _get_nc(n_tiles=N_TILES_FULL):
    if n_tiles not in _cache:
        _cache[n_tiles] = build_nc(n_tiles)
    return _cache[n_tiles]


def make_in_maps(x, y, weight, n_tiles=N_TILES_FULL):
    x = np.ascontiguousarray(np.asarray(x), dtype=np.float32)
    y = np.asarray(y)
    weight = np.asarray(weight)
    c_pad = n_tiles * TILE
    c_shard = min(C_SHARD, c_pad)
    ident = np.eye(128, dtype=ml_dtypes.bfloat16)
    wg = np.ascontiguousarray(weight[y].astype(np.float32))
    in_maps = []
    for i in range(N_CORES):
        ws = np.zeros((c_pad, D), dtype=np.float32)
        lo = i * c_shard
        hi = min(lo + c_shard, weight.shape[0])
        if hi > lo:
            ws[:hi - lo] = weight[lo:hi]
        in_maps.append({"x": x, "w": ws, "wg": wg, "ident": ident})
    return in_maps


def kernel(x, y, weight):
    nc = _get_nc()
    in_maps = make_in_maps(x, y, weight)
    res = run_bass_kernel_spmd(nc, in_maps, list(range(N_CORES)))
    out = res.results[0]["out"]
    return np.float32(out.reshape(-1)[0])


if __name__ == "__main__":
    rng = np.random.default_rng(0)
    x = rng.standard_normal((B, D)).astype(np.float32)
    y = rng.integers(0, C, size=(B,)).astype(np.int32)
    w = (rng.standard_normal((C, D)) * 0.008).astype(np.float32)
    print(kernel(x, y, w))
